# revision 1
# baseline (speedup 1.0000x reference)
"""Trainium2 Bass kernel for the CCM retrieval problem.

Reference computation (shapes: bs=64, N=1024, D=2048, H=128, C=65):
    z_x   = softmax(cos(all_f, emb)/T/sqrt(N))            [bs, N]
    hf    = head(all_f); hz = head(emb)                   [bs, H], [N, H]
    h1    = relu(BN(hf[b] @ A + b1 + hz[n] @ B))          [bs, N, H]
    y_zx  = softmax((h1 @ mix_w2 + mix_b2) @ clf_w + clf_b)  [bs, N, C]
    p_x   = softmax(sum_n cos(all_f, all_f)/T/sqrt(bs))   [bs]
    out   = z_x @ einsum('b,bnc->nc', p_x, y_zx)          [bs, C]

Device strategy: shard the queue axis N across 8 cores (128 rows each); bs
stays replicated so sum_x is core-local.  Host folds the BN affines into
weights, pre-multiplies mix_w2 @ clf_w (W2C) so the [bs,N,D] intermediate
never exists, pre-normalizes the embedding rows, and precomputes p_x.

Per core: head over [embT | all_fT] via 16 D-chunk matmuls; the mixer's
first layer collapses to alpha[h,b] (64 cols) and beta[h,n] (128 cols);
h1 slabs are produced per-b by a single fused relu(beta + alpha[:,b])
tensor_scalar in bf16, spread across DVE / ScalarE / GpSimd, feeding the
PE as the logits stationary.  exp on ScalarE; softmax row-sums, scale and
the p_x weighting on DVE/GpSimd; the b-sum rides PSUM accumulation of 64
[64,65] matmuls with a bf16 ez stationary.  The group pipeline is
software-interleaved so each engine always has the next group's work
queued.  Each core returns [64, 66]: columns 0:65 are exp(z-score) @
sum_x partial numerators, column 65 the z_x softmax denominator partial;
the host sums partials over cores and divides.
"""

import numpy as np
import ml_dtypes

import concourse.bass as bass
import concourse.tile as tile
from concourse import bacc, mybir
from concourse import bass_utils

F32 = mybir.dt.float32
BF16 = mybir.dt.bfloat16
AX = mybir.AxisListType
ALU = mybir.AluOpType
ACTF = mybir.ActivationFunctionType

T = 0.07
BN_EPS = 1e-5
BS, D, N, H, C = 64, 2048, 1024, 128, 65
NCORES = 8
NLOC = N // NCORES          # 128 queue rows per core
DCH = D // 128              # 16 contraction chunks
CP = 66                     # padded C stride (even -> 4B-aligned bf16 rows)
GRP = 16                    # b's per group (S/e2/accum granularity)
NG = BS // GRP              # 4 groups
HALF = 8                    # b's per 2-bank psum tile (4 per bank)

# tuning knobs: per 16-b group, the last U_GPS b's of the group run their
# relu(beta+alpha) on GpSimd, the U_ACT before those on ScalarE, the rest
# on DVE.  S_GPS lists the groups whose softmax row-sum runs on GpSimd
# (as an add-tree; gpsimd cannot reduce along X).  WEXP_DMA broadcasts
# the softmax scale [128,16] -> [128,16,CP] with a SBUF->SBUF DMA so the
# e2 multiply runs dense bf16 (2x mode) instead of a broadcast-AP 1x.
N_WARM_MM = 11
U_GPS = 0
U_ACT = 7
S_GPS = ()
WEXP_DMA = False
E2_GPS = False


def _build(with_c0: bool):
    nc = bacc.Bacc("TRN2", target_bir_lowering=False, debug=False)

    d_etf = nc.dram_tensor("etf", [128, DCH * 192], BF16, kind="ExternalInput")
    d_w1h = nc.dram_tensor("w1h", [128, DCH * H], BF16, kind="ExternalInput")
    d_pf = nc.dram_tensor("pf", [128, 130], F32, kind="ExternalInput")
    d_pb = nc.dram_tensor("pb", [128, 449], BF16, kind="ExternalInput")
    if with_c0:
        d_c0 = nc.dram_tensor("c0t", [1, C], BF16, kind="ExternalInput")
    d_out = nc.dram_tensor("out_nd", [BS, C + 1], F32, kind="ExternalOutput")

    with tile.TileContext(nc) as tc:
        with (
            tc.tile_pool(name="consts", bufs=1) as consts,
            tc.tile_pool(name="big", bufs=1) as bigp,
            tc.tile_pool(name="work", bufs=2) as work,
            tc.tile_pool(name="ubuf", bufs=8) as ubuf,
            tc.tile_pool(name="ebuf", bufs=2) as ebuf,
            tc.tile_pool(name="e2buf", bufs=2) as e2buf,
            tc.tile_pool(name="gbuf", bufs=2) as gbuf,
            tc.tile_pool(name="pbig", bufs=3, space="PSUM") as pbig,
            tc.tile_pool(name="phead", bufs=1, space="PSUM") as phead,
            tc.tile_pool(name="psmall", bufs=1, space="PSUM") as psmall,
        ):
            # warmup: exp table load at t=0; junk-fed PE spins through the
            # DMA window so HAM un-throttles before the real matmuls
            warm = consts.tile([1, 1], F32)
            nc.gpsimd.memset(warm, 0.0)
            wl = consts.tile([128, 128], BF16)
            nc.gpsimd.memset(wl, 0.0)
            wr = consts.tile([128, 512], BF16)
            nc.gpsimd.memset(wr, 0.0)
            for _ in range(N_WARM_MM):
                pw = pbig.tile([128, 1024], F32, tag="pb")
                nc.tensor.matmul(pw[:, 0:512], wl, wr, start=True, stop=True)

            # ---- input DMAs: consts first, then 4 chunk-pairs ----
            ones_col = consts.tile([128, 1], BF16)
            nc.gpsimd.memset(ones_col, 1.0)
            pf = consts.tile([128, 130], F32)
            nc.sync.dma_start(out=pf, in_=d_pf.ap())
            pb = consts.tile([128, 449], BF16)
            nc.scalar.dma_start(out=pb, in_=d_pb.ap())
            b1h_sb, cc_sb = pf[:, 0:1], pf[:, 1:2]
            nfs_b, px_b = pf[:, 2:66], pf[:, 66:130]
            wh2_sb, am_sb, bm_sb = pb[:, 0:128], pb[:, 128:256], pb[:, 256:384]
            w2c_sb = pb[:, 384:449]
            if with_c0:
                ones_row_bf = consts.tile([1, 128], BF16)
                nc.gpsimd.memset(ones_row_bf, 1.0)
                c0_sb = consts.tile([1, C], BF16)
                nc.sync.dma_start(out=c0_sb, in_=d_c0.ap())

            etf = bigp.tile([128, DCH, 192], BF16)
            w1h_sb = bigp.tile([128, DCH, H], BF16)
            etf_view = d_etf.ap().rearrange("p (i c) -> p i c", i=DCH)
            w1h_view = d_w1h.ap().rearrange("p (i h) -> p i h", i=DCH)
            for a, b in ((0, 2), (2, 4), (4, 8), (8, 16)):
                sl = slice(a, b)
                nc.sync.dma_start(out=etf[:, sl, :], in_=etf_view[:, sl, :])
                nc.scalar.dma_start(out=w1h_sb[:, sl, :], in_=w1h_view[:, sl, :])
            warm2 = consts.tile([1, 1], F32)
            nc.scalar.activation(warm2, warm, ACTF.Exp)

            # ---- head layer 1: X1 = relu(W1h.T @ [embT | all_fT] + b1h) ----
            xt = phead.tile([128, 192], F32, tag="ph")
            for i in range(DCH):
                nc.tensor.matmul(
                    xt, w1h_sb[:, i, :], etf[:, i, :], start=(i == 0),
                    stop=(i == DCH - 1), skip_group_check=True,
                )
            # z_x scores: slp[n, b] accumulates cos * nf / (T sqrt(N));
            # 4 chunks fill the x1-relu bubble, the rest go after alpha/beta
            slp = psmall.tile([NLOC, BS], F32, tag="ps")
            for i in range(4):
                nc.tensor.matmul(
                    slp, etf[:, i, 0:128], etf[:, i, 128:192], start=(i == 0),
                    stop=False, skip_group_check=True,
                )
            x1 = work.tile([128, 192], BF16)
            nc.scalar.activation(x1, xt, ACTF.Relu, bias=b1h_sb)
            # head layer 2 (head_b2 folded into cc)
            x2p = phead.tile([128, 192], F32, tag="ph")
            nc.tensor.matmul(x2p, wh2_sb, x1, skip_group_check=True)
            x2 = work.tile([128, 192], BF16)
            nc.vector.tensor_copy(x2, x2p)
            hz2 = x2[:, 0:128]
            hf2 = x2[:, 128:192]
            # mixer layer 1 collapses: alpha[h, b] (+cc), beta[h, n]
            abp = phead.tile([128, 192], F32, tag="ph")
            nc.tensor.matmul(abp[:, 0:64], am_sb, hf2, skip_group_check=True)
            nc.tensor.matmul(abp[:, 64:192], bm_sb, hz2, skip_group_check=True)
            for i in range(4, DCH):
                nc.tensor.matmul(
                    slp, etf[:, i, 0:128], etf[:, i, 128:192], start=False,
                    stop=(i == DCH - 1), skip_group_check=True,
                )
            alpha = work.tile([128, 64], F32)
            nc.scalar.activation(alpha, abp[:, 0:64], ACTF.Identity, bias=cc_sb)
            betaT = work.tile([128, 128], BF16)
            nc.vector.tensor_copy(betaT, abp[:, 64:192])
            # ez[n, b] = exp(score * nf / (T sqrt N)) in bf16 for the accum
            t3 = work.tile([NLOC, BS], F32)
            nc.vector.tensor_tensor(t3, slp, nfs_b, op=ALU.mult)
            ez = work.tile([NLOC, BS], BF16)
            nc.scalar.activation(ez, t3, ACTF.Exp)

            # ---- software-pipelined main loop over 4 groups of 16 b ----
            onp = psmall.tile([BS, C + 1], F32, tag="ps")
            e_tiles = [None] * NG
            st: dict = {}

            def emit_front(g):
                # u generation + logits matmuls + exp for group g
                e_g = ebuf.tile([128, GRP, CP], BF16, tag="e")
                e_tiles[g] = e_g
                if WEXP_DMA:
                    nc.gpsimd.memset(e_g[:, :, C:CP], 0.0)
                for h in range(2):
                    pg = pbig.tile([128, 1024], F32, tag="pb")
                    for j in range(HALF):
                        jg = HALF * h + j
                        b = GRP * g + jg
                        u = ubuf.tile([128, 128], BF16, tag="u")
                        a_col = alpha[:, b : b + 1]
                        if jg >= GRP - U_GPS:
                            nc.gpsimd.tensor_scalar(
                                u, betaT, a_col, 0.0, op0=ALU.add, op1=ALU.max
                            )
                        elif jg >= GRP - U_GPS - U_ACT:
                            nc.scalar.activation(
                                u, betaT, ACTF.Relu, bias=a_col
                            )
                        else:
                            nc.vector.tensor_scalar(
                                u, betaT, a_col, 0.0, op0=ALU.add, op1=ALU.max
                            )
                        off = 512 * (j // 4) + C * (j % 4)
                        sl = pg[:, off : off + C]
                        if with_c0:
                            nc.tensor.matmul(
                                sl, ones_row_bf, c0_sb, start=True, stop=False,
                                skip_group_check=True,
                            )
                            nc.tensor.matmul(
                                sl, u, w2c_sb, start=False, stop=True,
                                skip_group_check=True,
                            )
                        else:
                            nc.tensor.matmul(
                                sl, u, w2c_sb, start=True, stop=True,
                                skip_group_check=True,
                            )
                    pg_v = pg.rearrange("p (u x) -> p u x", u=2)[:, :, 0 : 4 * C]
                    pg_v = pg_v.rearrange("p u (j c) -> p u j c", c=C)
                    eh = e_g[:, HALF * h : HALF * (h + 1), 0:C]
                    nc.scalar.activation(
                        eh.rearrange("p (u j) c -> p u j c", u=2), pg_v, ACTF.Exp
                    )

            def emit_back_half(g, h):
                # per-half back phase for the last group: shortens the tail
                e_g = e_tiles[g]
                hs = slice(HALF * h, HALF * (h + 1))
                sg = work.tile([128, HALF], F32, tag=f"sgh{h}")
                nc.vector.reduce_sum(sg, e_g[:, hs, 0:C], axis=AX.X)
                rg = work.tile([128, HALF], F32, tag=f"rgh{h}")
                nc.vector.reciprocal(rg, sg)
                wg = work.tile([128, HALF], BF16, tag=f"wgh{h}")
                nc.vector.tensor_tensor(
                    wg, rg,
                    px_b[:, GRP * g + HALF * h : GRP * g + HALF * (h + 1)],
                    op=ALU.mult,
                )
                e2_g = e2buf.tile([128, HALF, CP], BF16, tag=f"e2h{h}")
                wv = wg.unsqueeze(2).broadcast_to([128, HALF, C])
                nc.vector.tensor_tensor(
                    e2_g[:, :, 0:C], e_g[:, hs, 0:C], wv, op=ALU.mult
                )
                for j in range(HALF):
                    b = GRP * g + HALF * h + j
                    nc.tensor.matmul(
                        onp[:, 0:C], ez, e2_g[:, j, 0:C],
                        start=(b == 0), stop=(b == BS - 1),
                        skip_group_check=True,
                    )

            def emit_back(g):
                # softmax denominators, p_x scale, e2 and psum accumulation
                if g == NG - 1:
                    emit_back_half(g, 0)
                    emit_back_half(g, 1)
                    return
                e_g = e_tiles[g]
                sg = work.tile([128, GRP], F32, tag="sg")
                if g in S_GPS:
                    # gpsimd cannot reduce along X: 64+1 add-tree instead
                    sc = gbuf.tile([128, GRP, 32], F32, tag="sc")
                    nc.gpsimd.tensor_tensor(
                        sc, e_g[:, :, 0:32], e_g[:, :, 32:64], op=ALU.add
                    )
                    nc.gpsimd.tensor_tensor(
                        sc[:, :, 0:16], sc[:, :, 0:16], sc[:, :, 16:32],
                        op=ALU.add,
                    )
                    nc.gpsimd.tensor_tensor(
                        sc[:, :, 0:8], sc[:, :, 0:8], sc[:, :, 8:16], op=ALU.add
                    )
                    nc.gpsimd.tensor_tensor(
                        sc[:, :, 0:4], sc[:, :, 0:4], sc[:, :, 4:8], op=ALU.add
                    )
                    nc.gpsimd.tensor_tensor(
                        sc[:, :, 0:2], sc[:, :, 0:2], sc[:, :, 2:4], op=ALU.add
                    )
                    nc.gpsimd.tensor_tensor(
                        sc[:, :, 0:1], sc[:, :, 0:1], sc[:, :, 1:2], op=ALU.add
                    )
                    nc.gpsimd.tensor_tensor(
                        sg.unsqueeze(2), sc[:, :, 0:1], e_g[:, :, 64:65],
                        op=ALU.add,
                    )
                else:
                    nc.vector.reduce_sum(sg, e_g[:, :, 0:C], axis=AX.X)
                rg = work.tile([128, GRP], F32, tag="rg")
                nc.vector.reciprocal(rg, sg)
                wg = work.tile([128, GRP], BF16, tag="wg")
                nc.vector.tensor_tensor(
                    wg, rg, px_b[:, GRP * g : GRP * (g + 1)], op=ALU.mult
                )
                e2_g = e2buf.tile([128, GRP, CP], BF16, tag="e2")
                if WEXP_DMA:
                    wx = gbuf.tile([128, GRP, CP], BF16, tag="wx")
                    nc.sync.dma_start(
                        out=wx, in_=wg.unsqueeze(2).broadcast_to([128, GRP, CP])
                    )
                    nc.vector.tensor_tensor(e2_g, e_g, wx, op=ALU.mult)
                elif E2_GPS:
                    wv = wg.unsqueeze(2).broadcast_to([128, GRP, C])
                    nc.gpsimd.tensor_tensor(
                        e2_g[:, :, 0:C], e_g[:, :, 0:C], wv, op=ALU.mult
                    )
                else:
                    wv = wg.unsqueeze(2).broadcast_to([128, GRP, C])
                    nc.vector.tensor_tensor(
                        e2_g[:, :, 0:C], e_g[:, :, 0:C], wv, op=ALU.mult
                    )
                for j in range(GRP):
                    b = GRP * g + j
                    nc.tensor.matmul(
                        onp[:, 0:C], ez, e2_g[:, j, 0:C],
                        start=(b == 0), stop=(b == BS - 1),
                        skip_group_check=True,
                    )

            emit_front(0)
            for g in range(1, NG):
                emit_front(g)
                emit_back(g - 1)
            emit_back(NG - 1)

            # ---- z_x denominator column, ship the partial result ----
            nc.tensor.matmul(onp[:, C : C + 1], ez, ones_col)
            on_s = work.tile([BS, C + 1], F32)
            nc.scalar.copy(on_s, onp)
            nc.sync.dma_start(out=d_out.ap(), in_=on_s)

    nc.compile()
    return nc


_CACHE: dict = {}
LAST_RESULTS = None  # BassKernelResults of the most recent run (for profiling)


def _get_nc(with_c0: bool):
    if with_c0 not in _CACHE:
        _CACHE[with_c0] = _build(with_c0)
    return _CACHE[with_c0]


def kernel(
    all_f, embedding, all_y,
    head_w1, head_b1, head_g, head_beta, head_rm, head_rv, head_w2, head_b2,
    mix_w1, mix_b1, mix_g, mix_beta, mix_rm, mix_rv, mix_w2, mix_b2,
    clf_w, clf_b,
):
    f64 = np.float64
    bf16 = ml_dtypes.bfloat16
    sh = head_g.astype(f64) / np.sqrt(head_rv.astype(f64) + BN_EPS)
    th = head_beta.astype(f64) - head_rm.astype(f64) * sh
    w1h = head_w1.astype(f64) * sh[None, :]
    b1h = (head_b1.astype(f64) * sh + th).astype(np.float32)[:, None]
    sm = mix_g.astype(f64) / np.sqrt(mix_rv.astype(f64) + BN_EPS)
    tm = mix_beta.astype(f64) - mix_rm.astype(f64) * sm
    am = mix_w1[:H].astype(f64) * sm[None, :]
    bm = mix_w1[H:].astype(f64) * sm[None, :]
    cm = mix_b1.astype(f64) * sm + tm
    ca = (head_b2.astype(f64) @ am + cm).astype(np.float32)[:, None]
    cb = (head_b2.astype(f64) @ bm).astype(np.float32)[:, None]
    w2c = (mix_w2.astype(f64) @ clf_w.astype(f64)).astype(bf16)
    c0 = (mix_b2.astype(f64) @ clf_w.astype(f64) + clf_b.astype(f64)).astype(
        np.float32
    )
    with_c0 = bool(np.any(c0 != 0.0))

    af = np.ascontiguousarray(all_f, dtype=np.float32)
    emb = np.ascontiguousarray(embedding, dtype=np.float32)
    # input-side host prep: row norms folded into the bf16 payloads, p_x
    nf = 1.0 / np.sqrt((af.astype(f64) ** 2).sum(axis=1))           # [bs]
    nfs = (nf / (T * np.sqrt(N))).astype(np.float32)
    gscore = ((af @ af.T).astype(f64) * nf[:, None] * nf[None, :]).sum(axis=1)
    gscore = gscore / (T * np.sqrt(BS))
    pe = np.exp(gscore - gscore.max())
    px = pe / pe.sum()                                              # [bs]
    aft = np.ascontiguousarray(af.T).astype(bf16)                   # [D, bs]

    pack_bf = np.empty((128, 449), dtype=bf16)
    pack_bf[:, 0:128] = np.ascontiguousarray(head_w2).astype(bf16)
    pack_bf[:, 128:256] = am.astype(bf16)
    pack_bf[:, 256:384] = bm.astype(bf16)
    pack_bf[:, 384:449] = w2c
    pack_f = np.empty((128, 130), dtype=np.float32)
    pack_f[:, 0:1] = b1h
    pack_f[:, 1:2] = ca + cb
    pack_f[:, 2:66] = np.broadcast_to(nfs[None, :], (128, BS))
    pack_f[:, 66:130] = np.broadcast_to(px[None, :].astype(np.float32), (128, BS))
    # w1h repack: [128, DCH*H], row p holds chunks i (w1h[i*128+p, :])
    w1h_p = np.ascontiguousarray(
        w1h.reshape(DCH, 128, H).transpose(1, 0, 2).reshape(128, DCH * H)
    ).astype(bf16)
    base = {"w1h": w1h_p, "pb": pack_bf, "pf": pack_f}
    if with_c0:
        base["c0t"] = c0[None, :].astype(bf16)

    ne = 1.0 / np.sqrt((emb.astype(f64) ** 2).sum(axis=1))          # [N]
    embn = (emb * ne[:, None].astype(np.float32)).astype(np.float64)

    in_maps = []
    for i in range(NCORES):
        shard = embn[i * NLOC : (i + 1) * NLOC]                     # [128, D]
        # etf packed [128, DCH*192]: row p, chunk i = [embT | afT] rows i*128+p
        etf3 = np.empty((128, DCH, 192), dtype=bf16)
        shard_t = shard.T.reshape(DCH, 128, NLOC)                   # [DCH,128,128]
        etf3[:, :, 0:128] = shard_t.transpose(1, 0, 2).astype(bf16)
        etf3[:, :, 128:192] = aft.reshape(DCH, 128, BS).transpose(1, 0, 2)
        in_maps.append(dict(base, etf=etf3.reshape(128, DCH * 192)))

    nc = _get_nc(with_c0)
    try:
        res = bass_utils.run_bass_kernel_spmd(
            nc, in_maps, core_ids=list(range(NCORES))
        )
    except Exception:
        # fresh NEFFs occasionally hit a transient NRT exec fault on their
        # first dispatch; one retry has always succeeded
        res = bass_utils.run_bass_kernel_spmd(
            nc, in_maps, core_ids=list(range(NCORES))
        )
    global LAST_RESULTS
    LAST_RESULTS = res
    parts = np.stack([r["out_nd"] for r in res.results], axis=0)  # [8, 64, 66]
    tot = parts.sum(axis=0)
    return (tot[:, :C] / tot[:, C : C + 1]).astype(np.float32)

